# revision 97
# baseline (speedup 1.0000x reference)
"""Trainium2 Bass kernel for relative-position multi-head attention.

Math (per batch element b, head h):
    k = key @ Wk.T + bk, q = query @ Wq.T + bq, v = value @ Wv.T + bv
    R = pe @ Wr.T + br                       # [2L, HID]; rpe[i,j] = R[j-i+L]
    A + C = (q + u_bias) @ k.T               # u folded into q
    B + D = skew((q + v_bias) @ R_h.T)       # skew: [i, dd] -> [i, j], dd = j-i+L
    score = (A+B+C+D)/sqrt(DH), mask keys j >= seq_len, softmax over j
    out = (attn @ v) @ Wf.T + bf

Device design (v2 — fp8 DoubleRow score path):
  - The whole score path runs in fp8e4 with MatmulPerfMode.DoubleRow
    (2 rows/cycle).  Host ships Wq/Wk and xq/xk as fp8 blobs with the
    contraction dim split in two column-interleaved groups ([64, 2, .]),
    so the q/k projections run at 0.5 cycles/row.  S2 (q@R.T) and the
    A (q@k.T) matmuls use DoubleRow with an all-zero second group
    (cost is out-free-size bound, so this still halves them), and the
    block-transpose matmuls use a host-shipped interleaved identity.
  - Scales: x*32, W*2048 -> psum q*65536; on-chip qu/qv/k are fp8 at
    x8 (copies rescale by 2^-13 and fold biases);  R is fp8 at x64, so
    S2 psum = S2*512, staged fp8; ident2 = I/8 makes the B path land at
    (A+B)*64 in the scores psum; exp uses scale 1/512.
  - v / final projections stay bf16 (error budget), out is bf16.
  - Compact skew via a DRAM round trip per head pair: one 4D-AP write
    [i, hh, it, dd-window] and per-half 4D-AP reads that deliver bt
    directly in the [64, 2(row-group), it, j] DoubleRow layout.
  - "scores transposed" layout [j (partitions), i (free)]: mask is a
    per-partition bias on the exp, denominator via a ones column in v.
  - One NeuronCore per batch element (data-parallel over batch).
"""

import os
import sys

try:
    import concourse.bass as bass  # noqa: F401
except ImportError:
    sys.path.insert(0, "/opt/trn_rl_repo")

import ml_dtypes
import numpy as np

import concourse.bass as bass
import concourse.tile as tile
from concourse import bacc, mybir
from concourse.bass_utils import run_bass_kernel_spmd

F32 = mybir.dt.float32
BF16 = mybir.dt.bfloat16
FP8 = mybir.dt.float8e4
AF = mybir.ActivationFunctionType
OP = mybir.AluOpType
DR = mybir.MatmulPerfMode.DoubleRow

B, L, HID, NH, DH = 8, 384, 512, 8, 64
DD = 2 * L          # 768 distinct relative positions
NT = L // 128       # 3 token tiles
CT = HID // 128     # 4 channel tiles
W = 512             # compact skew window width per i-tile
QCH = HID + L       # 896: per-kt chunk [W_kt | x_kt] in the v blob
NEG = -30000.0      # mask bias; exp(x*scale + NEG) == 0.0 in fp32

XS = 32.0           # host fp8 scale for xq/xk
WS = 2048.0         # host fp8 scale for Wq/Wk
QS = 8.0            # on-chip fp8 scale for qu/qv/k
RS = 64.0           # fp8 scale for R
PSQ_INV = QS / (XS * WS)          # 2^-13: psum -> x8 domain
EXP_SCALE = 1.0 / (QS * QS * 8.0)  # scores psum is (A+B)*64; /8 more for sqrt(DH)
IOTA2 = QS / RS                    # ident2 value: bt is S2*QS*RS, want B*QS^2

N_WARM = int(os.environ.get("K_NWARM", "6"))
PST_BUFS = int(os.environ.get("K_PST", "4"))
JUNK = os.environ.get("K_JUNK", "0") == "1"


def _build_program(skip_bias_rows: bool):
    nc = bacc.Bacc("TRN2", target_bir_lowering=False, debug=False, num_devices=8)

    def din(name, shape, dt=F32):
        return nc.dram_tensor(name, shape, dt, kind="ExternalInput").ap()

    # packed per-core inputs (channel-major activations, prepped on host)
    # q/k blobs: per kt a [2, 896] group-interleaved chunk [W2_kt | x2_kt]
    blob_q = din("blob_q", [64, CT * 2 * QCH], FP8)
    blob_k = din("blob_k", [64, CT * 2 * QCH], FP8)
    blob_v = din("blob_v", [128, CT * QCH], BF16)   # per kt: WvT_kt | vT_kt
    blob_f = din("blob_f", [128, CT * HID], BF16)   # WfT folded
    blob_i = din("blob_i", [128, 256], FP8)         # ident2 = I/8 interleaved
    blob_r = din("blob_r", [128, CT * DD], FP8)     # 64*R chan-major
    # folded cols [8(bq+u), 8(bq+v), 8bk, 0] then NT host-computed mask cols
    biases = din("biases", [128, CT * 4 + NT])
    # misc row 0: [ones(128) | bv(512) | bf(512) | pad]; rows 0-1 cols 0:128: sel
    misc = din("misc", [2, 1280], BF16)

    out = nc.dram_tensor("out", [L, HID], BF16, kind="ExternalOutput").ap()
    skews = [nc.dram_tensor(f"skew{p}", [2 * NT * 128 * W + 512], FP8) for p in range(4)]

    with tile.TileContext(nc) as tc, nc.allow_low_precision(
        reason="bf16/fp8 mixed precision is intentional; accumulation is fp32 PSUM"
    ):
        _body(tc, locals())
    nc.compile()
    return nc


def _body(tc, io):
    nc = tc.nc
    skip_bias_rows = io["skip_bias_rows"]

    from contextlib import ExitStack

    with ExitStack() as ctx:
        consts = ctx.enter_context(tc.tile_pool(name="consts", bufs=1))
        work = ctx.enter_context(tc.tile_pool(name="work", bufs=1))
        s2_pool = ctx.enter_context(tc.tile_pool(name="s2b", bufs=4))
        exp_pool = ctx.enter_context(tc.tile_pool(name="exps", bufs=1))
        osb_pool = ctx.enter_context(tc.tile_pool(name="osb", bufs=3))
        # PSUM is phased: psA (2 banks) global; psS2 (6) lives only through
        # the S2 phase, then its banks are reused by psT (4) + psV (2).
        psA = ctx.enter_context(tc.tile_pool(name="psA", bufs=2, space="PSUM"))
        s2_ctx = ExitStack()
        psS2 = s2_ctx.enter_context(tc.tile_pool(name="psS2", bufs=5, space="PSUM"))
        psT = psV = None  # opened after the S2 phase

        # ---- input DMAs.  DMA_ENGINES is a serial resource: intended
        # transfer order is q, bcols, R, ident, v, k, w0, r0, w1, r1, ..., f,
        # outs.  k is dep-delayed behind bcols so its HWDGE slot doesn't jump
        # ahead of the latency-critical small transfers.
        bq = consts.tile([64, CT, 2, QCH], FP8, tag="bq", name="bq")
        _bqv = io["blob_q"].rearrange("p (a g n) -> p a g n", g=2, n=QCH)
        nc.sync.dma_start(out=bq, in_=_bqv)

        bcols = consts.tile([128, CT * 4 + NT], F32, tag="biases", name="biases")
        nc.sync.dma_start(out=bcols, in_=io["biases"])
        masks = [bcols[:, CT * 4 + jt : CT * 4 + jt + 1] for jt in range(NT)]

        # Layout: [ident2 (256) | per pair a: R_a (DD) then zeros (DD) | pad].
        # The trailing pad keeps the rslice views formally in-bounds.
        BRW = 256 + CT * 2 * DD + 256
        br_ = consts.tile([128, BRW], FP8, tag="br", name="br")
        brv = br_[:, 256 : 256 + CT * 2 * DD].rearrange(
            "p (a g n) -> p a g n", g=2, n=DD
        )
        nc.sync.dma_start(
            out=brv[:, :, 0, :],
            in_=io["blob_r"].rearrange("p (a n) -> p a n", n=DD),
        )
        nc.sync.dma_start(out=br_[:, 0:256], in_=io["blob_i"])

        bv_ = consts.tile([128, CT, QCH], BF16, tag="bv", name="bv")
        nc.vector.tensor_copy(out=bv_[0:1, 0, 0:1], in_=bcols[0:1, 0:1])
        nc.scalar.dma_start(
            out=bv_, in_=io["blob_v"].rearrange("p (a n) -> p a n", n=QCH)
        )

        bk = consts.tile([64, CT, 2, QCH], FP8, tag="bk", name="bk")
        _bkv = io["blob_k"].rearrange("p (a g n) -> p a g n", g=2, n=QCH)
        nc.vector.tensor_copy(out=bk[0:1, 0, 0, 0:1], in_=bcols[0:1, 0:1])
        nc.scalar.dma_start(out=bk, in_=_bkv)

        # zero the second (padding) group of each R pair block + tail pad
        for a in range(CT):
            nc.gpsimd.memset(br_[:, 256 + a * 2 * DD + DD : 256 + (a + 1) * 2 * DD], 0.0)
        nc.gpsimd.memset(br_[:, 256 + CT * 2 * DD :], 0.0)

        ident2 = br_[:, 0:256].rearrange("p (g n) -> p g n", g=2)

        def rslice(h, it):  # 64*R window for head h, i-tile it: [64, 2, W], g1=0
            a, half = h // 2, (h % 2) * 64
            base = 256 + a * 2 * DD + 128 * (2 - it)
            return br_[half : half + 64, base : base + 2 * DD].rearrange(
                "p (g n) -> p g n", g=2
            )[:, :, 0:W]

        def wslice_v(kt):  # Wv block [128, 512]
            return bv_[:, kt, 0:HID]

        def xslice_v(kt, sl):  # v activation block [128, |sl|]
            return bv_[:, kt, HID + sl.start : HID + sl.stop]

        # ---- PE warm-up: ramp the clock while input DMAs are in flight.
        # The tiny activation pre-loads the ACT function table (1.3us) into
        # the idle pre-data window so the first real copy isn't delayed.
        wtile = consts.tile([128, 128], BF16, tag="wtile", name="wtile")
        nc.vector.memset(wtile, 0.0)
        dtile = consts.tile([1, 1], BF16, tag="dtile", name="dtile")
        nc.gpsimd.memset(dtile, 0.0)
        nc.scalar.activation(out=dtile, in_=dtile, func=AF.Exp, scale=1.0)
        for w in range(4 * N_WARM):
            psw = psA.tile([128, 128], F32, tag="psA", name=f"warm{w}",
                           padded_shape=[128, 512])
            nc.tensor.matmul(psw, wtile, wtile, start=True, stop=True)

        miscs = consts.tile([2, 1280], BF16, tag="misc", name="misc")
        nc.sync.dma_start(out=miscs, in_=io["misc"])
        ones_row = miscs[0:1, 128:256]
        bv_row = miscs[0:1, 256:768]
        bf_row = miscs[0:1, 768:1280]
        ones64 = consts.tile([128, 64], BF16, tag="ones64", name="ones64")
        nc.gpsimd.memset(ones64, 1.0)

        if JUNK:
            # junk-tail writes: make the full-rate re-aligned reads below
            # end in initialized bytes (only needed under CoreSim checking).
            junk = consts.tile([1, 512], FP8, tag="junk", name="junk")
            nc.vector.memset(junk, 0.0)
            for p in range(4):
                nc.gpsimd.dma_start(
                    out=bass.AP(
                        tensor=io["skews"][p], offset=2 * NT * 128 * W,
                        ap=[[512, 1], [1, 512]],
                    ),
                    in_=junk,
                )
        # qv_cm[mt]: [128, 2, L] fp8 [qv | Z]: S2's DoubleRow stationary; the
        # moving R window's group 1 is zero, the Z block just keeps group-1
        # data finite (no NaN).  qu/k stay bf16 for the A-path's precision.
        qv_cm = [
            work.tile([128, 2, L], FP8, tag=f"qv{mt}", name=f"qv{mt}")
            for mt in range(CT)
        ]
        qu_cm = [
            work.tile([128, L], BF16, tag=f"qu{mt}", name=f"qu{mt}")
            for mt in range(CT)
        ]
        k_cm = [
            work.tile([128, L], BF16, tag=f"k{mt}", name=f"k{mt}")
            for mt in range(CT)
        ]
        for mt in range(CT):
            nc.gpsimd.memset(qv_cm[mt][:, 1, :], 0.0)

        def qv2(mt, hs, isl):  # [64, 2, 128] stationary for S2 (g1 = zeros)
            return qv_cm[mt][hs, 0:2, isl]

        v_ext = [None] * NT
        ot_cm = [
            work.tile([128, L], BF16, tag=f"ot_cm{mt}", name=f"ot_cm{mt}")
            for mt in range(CT)
        ]
        s2_t = [None] * 4
        bt_t = [None] * 4
        exps_t = [None] * NH
        ppv_t = [None] * NH

        def proj_q(mt):
            ms = slice(mt * 128, (mt + 1) * 128)
            ps = psA.tile([128, 512], F32, tag="psA", name="psq")
            for kt in range(CT):
                nc.tensor.matmul(
                    ps[:, 0:L], bq[:, kt, :, ms], bq[:, kt, :, HID : HID + L],
                    start=(kt == 0), stop=(kt == CT - 1), perf_mode=DR,
                )
            # qu (ACT, bf16 at x8), qv (DVE, fp8 at x8)
            nc.scalar.activation(
                out=qu_cm[mt], in_=ps[:, 0:L], func=AF.Identity,
                bias=bcols[:, mt * 4 : mt * 4 + 1], scale=PSQ_INV,
            )
            nc.vector.tensor_scalar(
                out=qv_cm[mt][:, 0, :], in0=ps[:, 0:L],
                scalar1=PSQ_INV, scalar2=bcols[:, mt * 4 + 1 : mt * 4 + 2],
                op0=OP.mult, op1=OP.add,
            )

        def proj_k(mt):
            ms = slice(mt * 128, (mt + 1) * 128)
            ps = psA.tile([128, 512], F32, tag="psA", name="psk")
            for kt in range(CT):
                nc.tensor.matmul(
                    ps[:, 0:L], bk[:, kt, :, ms], bk[:, kt, :, HID : HID + L],
                    start=(kt == 0), stop=(kt == CT - 1), perf_mode=DR,
                )
            nc.vector.tensor_scalar(
                out=k_cm[mt], in0=ps[:, 0:L],
                scalar1=PSQ_INV, scalar2=bcols[:, mt * 4 + 2 : mt * 4 + 3],
                op0=OP.mult, op1=OP.add,
            )

        def proj_v(it):
            # v token-major, packed per head: [64 v cols][1 ones][1 pad] x 8.
            # The ones column folds the softmax denominator into attn @ v.
            isl = slice(it * 128, (it + 1) * 128)
            ps = psA.tile([128, 512], F32, tag="psA", name="psv")
            for kt in range(CT):
                nc.tensor.matmul(
                    ps, xslice_v(kt, isl), wslice_v(kt).opt(),
                    start=(kt == 0), stop=(kt == CT - 1) and skip_bias_rows,
                )
            if not skip_bias_rows:
                nc.tensor.matmul(ps, ones_row, bv_row, start=False, stop=True)
            t = work.tile([128, NH, 66], BF16, tag=f"v_ext{it}", name=f"v_ext{it}")
            nc.vector.tensor_copy(
                out=t[:, :, 0:64], in_=ps.rearrange("p (h d) -> p h d", h=NH)
            )
            nc.vector.memset(t[:, :, 64:65], 1.0)
            v_ext[it] = t

        def s2_pair(p):
            """S2 = (q + v_bias)*8 @ (64*R_h).T over compact windows for heads
            2p, 2p+1 into one SBUF staging tile (fp8, value S2*512)."""
            s2b = s2_pool.tile([128, 2, NT, W], FP8, tag="s2b", name="s2b")
            for hh in range(2):
                h = 2 * p + hh
                mt, half = h // 2, (h % 2) * 64
                hs = slice(half, half + 64)
                for it in range(NT):
                    isl = slice(it * 128, (it + 1) * 128)
                    ps2 = psS2.tile([128, W], F32, tag="s2", name="ps2")
                    nc.tensor.matmul(
                        ps2, qv2(mt, hs, isl), rslice(h, it),
                        start=True, stop=True, perf_mode=DR,
                        tile_position=(half, 0),
                    )
                    eng = (nc.vector, nc.scalar,
                           nc.vector if hh == 0 else nc.scalar)[it]
                    if eng is nc.scalar:
                        eng.copy(out=s2b[:, hh, it, :], in_=ps2)
                    else:
                        eng.tensor_copy(out=s2b[:, hh, it, :], in_=ps2)
            s2_t[p] = s2b

        def skew_write(p):
            sk = io["skews"][p]
            nc.sync.dma_start(
                out=bass.AP(
                    tensor=sk, offset=0,
                    ap=[[W, 128], [NT * 128 * W, 2], [128 * W, NT], [1, W]],
                ),
                in_=s2_t[p],
            )

        # Persistent bt tiles [128, 2*NT+1, W], one per pair: blocks 0..5 are
        # (hh, it) flat, filled by the skew reads; block 6 is a finite pad so
        # every DoubleRow stationary can take (block, block+1).  The identity
        # moving operand's second group is zero, so group-1 data is unused.
        bt_tiles = [
            work.tile([128, 2 * NT + 1, W], FP8, tag=f"bt{p}", name=f"bt{p}")
            for p in range(4)
        ]
        for p in range(4):
            nc.gpsimd.memset(bt_tiles[p][:, 2 * NT, :], 0.0)

        def skew_read(p):
            # re-aligned, per head half: bt[li, hh*NT+it, j] =
            #   scratch[hh*NT*128*W + it*128*W + li*(W-1) + j + 128]
            # Same sync queue as the writes: the in-order queue + serial DMA
            # engines give the RAW ordering without a semaphore round trip.
            # (Two reads, not one merged: the first half's earlier completion
            # lets that head's transpose matmuls start ~0.5us sooner.)
            for hh in range(2):
                nc.sync.dma_start(
                    out=bt_tiles[p][:, hh * NT : hh * NT + NT, :],
                    in_=bass.AP(
                        tensor=io["skews"][p], offset=hh * NT * 128 * W + 128,
                        ap=[[W - 1, 128], [128 * W, NT], [1, W]],
                    ),
                )
            bt_t[p] = bt_tiles[p]

        def scores(h):
            """Scores psum = (A+B)*64 in [j, i] layout: B via DoubleRow fp8
            block-transpose matmuls against ident2, A via DoubleRow fp8
            (zero-padded group); masked exp (scale 1/512 folds everything)."""
            mt, half = h // 2, (h % 2) * 64
            hs = slice(half, half + 64)
            bt = bt_t[h // 2]
            hh = h % 2
            exps = exp_pool.tile([128, NT, L], BF16, tag=f"exps{h % 4}", name="exps")
            for jt in range(NT):
                jsl = slice(jt * 128, (jt + 1) * 128)
                pst = psT.tile([128, L], F32, tag="pst", name="pst")
                for it in range(NT):
                    blk = hh * NT + it
                    nc.tensor.matmul(
                        pst[:, it * 128 : (it + 1) * 128],
                        bt[:, blk : blk + 2, jsl], ident2,
                        start=True, stop=False, perf_mode=DR,
                    )
                nc.tensor.matmul(
                    pst, k_cm[mt][hs, jsl], qu_cm[mt][hs, :],
                    start=False, stop=True, tile_position=(half, 0),
                )
                nc.scalar.activation(
                    out=exps[:, jt, :], in_=pst, func=AF.Exp,
                    bias=masks[jt], scale=EXP_SCALE,
                )
            exps_t[h] = exps

        def attn_v(h):
            # attn @ v in two accumulation groups of one [128, L] psum tile:
            # rows 0..63 accumulate the denominator REPLICATED 64x (all-ones
            # stationary), rows 64..127 the head output.  One DVE reciprocal
            # [64, L] then yields the broadcast 1/denom directly -- no PE
            # broadcast matmul and no extra psum->sbuf copy.
            ppv = psV.tile([128, L], F32, tag=f"ppv{h % 2}", name="ppv")
            for kt in range(NT):
                nc.tensor.matmul(
                    ppv[0:64, :], ones64, exps_t[h][:, kt, :],
                    start=(kt == 0), stop=(kt == NT - 1),
                )
            rrb = work.tile([64, L], F32, tag=f"rr_{h % 4}", name="rrb")
            nc.vector.reciprocal(out=rrb, in_=ppv[0:64, :])
            rrb_t[h] = rrb
            for kt in range(NT):
                nc.tensor.matmul(
                    ppv[64:128, :], v_ext[kt][:, h, 0:64].opt(),
                    exps_t[h][:, kt, :],
                    start=(kt == 0), stop=(kt == NT - 1),
                )
            ppv_t[h] = ppv

        rrb_t = [None] * NH

        def norm_pair(mt):
            """Normalize heads 2mt, 2mt+1 into ot_cm."""
            h0, h1 = 2 * mt, 2 * mt + 1
            nc.vector.tensor_tensor(
                out=ot_cm[mt][0:64, :], in0=ppv_t[h0][64:128, :],
                in1=rrb_t[h0], op=OP.mult,
            )
            nc.vector.tensor_tensor(
                out=ot_cm[mt][64:128, :], in0=ppv_t[h1][64:128, :],
                in1=rrb_t[h1], op=OP.mult,
            )

        # ---- pipeline ----
        # All q projections first (their fp8 copies must lead the DVE/ACT
        # queues), then S2 pairs whose staging copies pace the round trips.
        # Skew writes+reads interleave w0,r0,w1,r1,... on one queue so the
        # serial DMA engines drain them in exactly that order.
        proj_q(0)
        proj_q(1)
        proj_q(2)
        proj_q(3)
        s2_pair(0)
        skew_write(0)
        skew_read(0)
        s2_pair(1)
        skew_write(1)
        skew_read(1)
        s2_pair(2)
        skew_write(2)
        skew_read(2)
        s2_pair(3)
        skew_write(3)
        skew_read(3)

        # f blob load: dep-delayed (via the dummy copy below) so the
        # scheduler cannot hoist its transfer into the round-trip window.
        bf_ = consts.tile([128, CT, HID], BF16, tag="bf", name="bf")
        nc.vector.tensor_copy(out=bf_[0:1, 0, 0:1], in_=bt_t[2][0:1, NT, 0:1])
        nc.scalar.dma_start(
            out=bf_, in_=io["blob_f"].rearrange("p (a n) -> p a n", n=HID)
        )

        def wfull_f(kt):
            return bf_[:, kt, 0:HID]

        for mt in range(CT):
            proj_k(mt)

        # phase 2: psA+psS2's banks are handed to the scores/final pools.
        s2_ctx.close()
        psT = ctx.enter_context(tc.tile_pool(name="psT", bufs=PST_BUFS, space="PSUM"))
        psV = ctx.enter_context(tc.tile_pool(name="psV", bufs=1, space="PSUM"))

        psf01 = [None, None, None]

        def f_partial(it):
            # accumulate kt0-2 early (ot_cm[0..2] long ready)
            isl = slice(it * 128, (it + 1) * 128)
            if it < 2:
                ps = psA.tile([128, 512], F32, tag="psA", name="psf")
            else:
                ps = psT.tile([128, 512], F32, tag="pst", name="psf2")
            for kt in range(NT):
                nc.tensor.matmul(
                    ps, ot_cm[kt][:, isl], wfull_f(kt).opt(),
                    start=(kt == 0), stop=False,
                )
            psf01[it] = ps

        def f_finish(it, ps, kts):
            isl = slice(it * 128, (it + 1) * 128)
            for kt in kts:
                nc.tensor.matmul(
                    ps, ot_cm[kt][:, isl], wfull_f(kt).opt(),
                    start=(kt == 0), stop=(kt == CT - 1) and skip_bias_rows,
                )
            if not skip_bias_rows:
                nc.tensor.matmul(ps, ones_row, bf_row, start=False, stop=True)
            osb = osb_pool.tile([128, 512], BF16, tag="osb", name="osb")
            if it == 1:
                nc.vector.tensor_copy(out=osb, in_=ps)
            else:
                nc.scalar.copy(out=osb, in_=ps)
            (nc.scalar if it == 1 else nc.sync).dma_start(
                out=io["out"][isl, :], in_=osb
            )

        scores(0)
        scores(1)
        for it in range(NT):
            proj_v(it)
        attn_v(0)
        attn_v(1)
        for mt in range(1, CT):
            scores(2 * mt)
            scores(2 * mt + 1)
            norm_pair(mt - 1)
            attn_v(2 * mt)
            attn_v(2 * mt + 1)
        f_partial(0)
        f_partial(1)
        f_partial(2)
        # tail: normalize pair 3 per it-column slice so each output tile's
        # final accumulation + writeback fires as soon as its slice lands.
        for it in range(NT):
            isl = slice(it * 128, (it + 1) * 128)
            nc.vector.tensor_tensor(
                out=ot_cm[3][0:64, isl], in0=ppv_t[6][64:128, isl],
                in1=rrb_t[6][:, isl], op=OP.mult,
            )
            nc.vector.tensor_tensor(
                out=ot_cm[3][64:128, isl], in0=ppv_t[7][64:128, isl],
                in1=rrb_t[7][:, isl], op=OP.mult,
            )
            f_finish(it, psf01[it], [CT - 1])


_CACHE = {}


def _get_nc(skip_bias_rows: bool):
    key = skip_bias_rows
    if key not in _CACHE:
        _CACHE[key] = _build_program(skip_bias_rows)
    return _CACHE[key]


def _fold(a):
    """[HID, N] -> [128, CT, N] channel-folded: row p, block a covers
    DRAM row a*128+p."""
    n = a.shape[1]
    return np.ascontiguousarray(a.reshape(CT, 128, n).transpose(1, 0, 2))


def _fold_dr(a):
    """[HID, N] -> [64, CT, 2, N] DoubleRow-folded: row p, chunk kt, group g
    covers DRAM row 128*kt + 64*g + p."""
    n = a.shape[1]
    return np.ascontiguousarray(a.reshape(CT, 2, 64, n).transpose(2, 0, 1, 3))


def _f8(x, s):
    f8 = ml_dtypes.float8_e4m3
    return np.clip(np.asarray(x, np.float32) * s, -240.0, 240.0).astype(f8)


def prep_in_maps(inputs):
    """Host-side sharding + layout marshaling. Returns (in_maps, skip_bias_rows)."""
    f = np.float32
    bf = ml_dtypes.bfloat16
    g = {k: np.asarray(v) for k, v in inputs.items()}

    # R = pe @ Wr.T + br computed on host; shipped channel-major, x64, fp8.
    R = (g["pe"].astype(f) @ g["Wr"].astype(f).T) + g["br"].astype(f)  # [DD, HID]
    blob_r = np.ascontiguousarray(
        _fold(np.ascontiguousarray(R.T)).reshape(128, CT * DD) * RS
    )
    blob_r = _f8(blob_r, 1.0)

    # ident2: [128, 2, 128] -> [128, 256]; group 0 = I * IOTA2, group 1 = 0
    # (the zero group pairs with the bt tiles' zero padding group).
    ident2 = np.zeros((128, 2, 128), f)
    ident2[:, 0, :] = np.eye(128, dtype=f) * IOTA2
    blob_i = _f8(ident2.reshape(128, 256), 1.0)

    wq = _fold_dr(np.ascontiguousarray(g["Wq"].astype(f).T))  # [64, CT, 2, HID]
    wk = _fold_dr(np.ascontiguousarray(g["Wk"].astype(f).T))
    wv = _fold(np.ascontiguousarray(g["Wv"].astype(f).T))
    wf = _fold(np.ascontiguousarray(g["Wf"].astype(f).T))

    biases = np.stack(
        [
            (g["bq"].astype(f) + g["u_bias"].astype(f).reshape(-1)) * QS,
            (g["bq"].astype(f) + g["v_bias"].astype(f).reshape(-1)) * QS,
            g["bk"].astype(f) * QS,
            np.zeros(HID, f),
        ],
        axis=1,
    )  # [HID, 4] -> folded [128, CT*4]
    biases = np.ascontiguousarray(
        biases.reshape(CT, 128, 4).transpose(1, 0, 2).reshape(128, CT * 4)
    )

    misc = np.zeros((2, 1280), f)
    misc[0, 0:64] = 1.0     # sel row 0
    misc[1, 64:128] = 1.0   # sel row 1
    misc[0, 128:256] = 1.0  # ones row
    misc[0, 256:768] = g["bv"].astype(f)
    misc[0, 768:1280] = g["bf"].astype(f)
    misc = misc.astype(bf)

    skip_bias_rows = not (np.any(g["bv"]) or np.any(g["bf"]))

    shared = {
        "blob_r": blob_r,
        "blob_i": blob_i,
        "misc": misc,
        "blob_f": np.ascontiguousarray(wf.reshape(128, CT * HID)).astype(bf),
    }

    wq8 = _f8(wq, WS)
    wk8 = _f8(wk, WS)

    def qblob(w8, xt8):  # [64, CT, 2, 896] -> [64, CT*2*896]
        return np.ascontiguousarray(
            np.concatenate([w8, xt8], axis=3).reshape(64, CT * 2 * QCH)
        )

    def vblob(wt, xt):  # per-kt chunks [W_kt | x_kt] -> [128, CT*QCH]
        return np.ascontiguousarray(
            np.concatenate([wt, xt], axis=2).reshape(128, CT * QCH)
        ).astype(bf)

    seq = np.asarray(g["seq_len"]).astype(np.int64)
    jidx = np.arange(L)
    in_maps = []
    for b in range(B):
        m = dict(shared)
        qT8 = _f8(_fold_dr(np.ascontiguousarray(g["query"][b].astype(f).T)), XS)
        kT8 = _f8(_fold_dr(np.ascontiguousarray(g["key"][b].astype(f).T)), XS)
        vT = _fold(np.ascontiguousarray(g["value"][b].astype(f).T))
        m["blob_q"] = qblob(wq8, qT8)
        m["blob_k"] = qblob(wk8, kT8)
        m["blob_v"] = vblob(wv, vT)
        # per-core mask columns: NEG where key index >= seq_len
        mcols = np.where(jidx >= seq[b], np.float32(NEG), np.float32(0.0))
        m["biases"] = np.concatenate(
            [biases, mcols.reshape(NT, 128).T.astype(f)], axis=1
        )
        in_maps.append(m)
    return in_maps, skip_bias_rows


def kernel(**inputs) -> np.ndarray:
    in_maps, skip_bias_rows = prep_in_maps(inputs)
    nc = _get_nc(skip_bias_rows)
    res = run_bass_kernel_spmd(nc, in_maps, list(range(B)))
    return np.stack([res.results[c]["out"] for c in range(B)]).astype(np.float32)
